# revision 18
# baseline (speedup 1.0000x reference)
"""NemotronH Top-k MoE router on 8 Trainium2 NeuronCores.

Token-parallel: 16384 tokens sharded 2048/core; router weight replicated.

Matmul in [t, e] layout with a fused moving operand: for each 128-token
block, PSUM out[t=128, 512] holds main logits in cols 0:256 and the
correction in cols 256:512.  Pass 1 streams the combined [wh|wl] weight
(N=512) with the hi-plane chunk stationary (computes h*wh and h*wl in one
sweep); pass 2 streams wh (N=256) with the lo-plane chunk stationary and
accumulates l*wh onto the correction half.  Same FLOPs as a 3-pass
scheme but 2/3 the LDWEIGHTS and one PSUM bank per block.  Block 0 runs
unfused (wh-only passes first) so the PE can start before the lo-plane
weights arrive; weight DMA streams the wh plane first for the same
reason, and block DMAs are plane-split so a block's main pass only waits
on its hi half.

Precision: fp16 hi/lo planes, x = h + l/2048 (22 mantissa bits); products
in fp22, fp32 PSUM accumulation; logits = main + corr/2048.  All
selection comparisons run in fp32.

Routing per block (tokens on partitions, experts on free axis): sigmoid,
bias add, group top-2 via reduce_max + match_replace, group top-4 via
max8 threshold, top-8 via max8 + max_index, per-slot score extraction via
is_equal(iota) + accumulate, normalize, scale.

Timing programs (reps > 1) unroll the loop body 2x so the For_i
all-engine barrier only fires every other invocation and consecutive
bodies pipeline across the seam.
"""
import sys
sys.path.insert(0, "/opt/trn_rl_repo")

import numpy as np

from concourse import bacc, tile, mybir
from concourse.bass_utils import run_bass_kernel_spmd

F32 = mybir.dt.float32
F16 = mybir.dt.float16
U16 = mybir.dt.uint16
I32 = mybir.dt.int32
AF = mybir.ActivationFunctionType
ALU = mybir.AluOpType

T_TOTAL = 16384
H = 4096
E = 256
G, GS = 8, 32
TOP_K = 8
N_CORES = 8
T_CORE = T_TOTAL // N_CORES      # 2048
TB = 128                         # tokens per block (PSUM partition dim)
NB = T_CORE // TB                # 16 blocks
KC = H // 128                    # 32 k-chunks
S = 2048.0                       # lo-plane scale (2^11)
ROUTED_SCALING = 2.5


def build_program(reps=1, staggered_reset=False, unroll=1):
    nc = bacc.Bacc("TRN2", target_bir_lowering=False)
    # host pre-layout: [blk, p, plane, c, tb]; per-plane partition lines are
    # 8 KB contiguous so hi/lo can stream as separate DMAs
    hst_c = nc.dram_tensor("hst_c", [NB, 128, 2, KC, TB], F16, kind="ExternalInput")
    # combined weights [p, c, (wh|wl)]
    wt_d = nc.dram_tensor("wt_hl", [128, KC, 2 * E], F16, kind="ExternalInput")
    bias_d = nc.dram_tensor("bias_bc", [128, E], F32, kind="ExternalInput")
    iota_d = nc.dram_tensor("iota_bc", [128, E], F32, kind="ExternalInput")
    idx_out = nc.dram_tensor("idx_out", [T_CORE, TOP_K], I32, kind="ExternalOutput")
    w_out = nc.dram_tensor("w_out", [T_CORE, TOP_K], F32, kind="ExternalOutput")

    with tile.TileContext(nc) as tc:
        with (
            tc.tile_pool(name="const", bufs=1) as cpool,
            tc.tile_pool(name="hs", bufs=4) as hspool,
            tc.tile_pool(name="rt", bufs=2) as rt,
            tc.tile_pool(name="outp", bufs=2) as outp,
            tc.tile_pool(name="ps", bufs=4, space="PSUM") as ps,
        ):
            wt = cpool.tile([128, KC, 2 * E], F16, name="wt")
            bias_t = cpool.tile([128, E], F32)
            iota_t = cpool.tile([128, E], F32)
            # weights on the ACT HWDGE ring (hidden goes on the SP ring).
            # hi plane first in chunk slices (512 B runs -> full-rate DMA),
            # then bias/iota, then the lo plane: block 0 is scheduled to not
            # need wl until ~2 passes in.
            for c0, c1 in ((0, 2), (2, 8), (8, 20), (20, KC)):
                nc.scalar.dma_start(wt[:, c0:c1, 0:E], wt_d[:, c0:c1, 0:E])
            nc.scalar.dma_start(bias_t[:], bias_d[:])
            for c0, c1 in ((0, 8), (8, 20), (20, KC)):
                nc.scalar.dma_start(wt[:, c0:c1, E:2 * E], wt_d[:, c0:c1, E:2 * E])
            nc.scalar.dma_start(iota_t[:], iota_d[:])

            def alloc_block(blk):
                return hspool.tile([128, 2, KC, TB], F16, tag="xc",
                                   name=f"xc{blk}")

            def dma_hi(blk, xc, sliced=False):
                if sliced:
                    for c0, c1 in ((0, 2), (2, 8), (8, 20), (20, KC)):
                        nc.sync.dma_start(xc[:, 0, c0:c1, :],
                                          hst_c[blk, :, 0, c0:c1, :])
                else:
                    nc.sync.dma_start(xc[:, 0], hst_c[blk, :, 0])

            def dma_lo(blk, xc):
                nc.sync.dma_start(xc[:, 1], hst_c[blk, :, 1])

            def dma_block(blk):
                xc = alloc_block(blk)
                dma_hi(blk, xc)
                dma_lo(blk, xc)
                return xc

            def mm_main_pass(xc, acc):
                # main into cols 0:256 (wh only).  per-element has_written
                # bits make the first write to each region an overwrite.
                for c in range(KC):
                    nc.tensor.matmul(
                        acc[:, 0:E], xc[:, 0, c, :], wt[:, c, 0:E],
                        start=(c == 0), stop=False)

            def mm_lwh_pass(xc, acc):
                for c in range(KC):
                    nc.tensor.matmul(
                        acc[:, E:2 * E], xc[:, 1, c, :], wt[:, c, 0:E],
                        start=False, stop=False)

            def mm_hwl_pass(xc, acc):
                for c in range(KC):
                    nc.tensor.matmul(
                        acc[:, E:2 * E], xc[:, 0, c, :], wt[:, c, E:2 * E],
                        start=False, stop=(c == KC - 1))

            def mm_fused(xc, acc):
                # pass 1: acc[:, 0:256] += xh_c.T @ wh_c
                #         acc[:, 256:512] += xh_c.T @ wl_c
                for c in range(KC):
                    nc.tensor.matmul(
                        acc[:, 0:2 * E], xc[:, 0, c, :], wt[:, c, :],
                        start=(c == 0), stop=False)
                # pass 2: acc[:, 256:512] += xl_c.T @ wh_c
                for c in range(KC):
                    nc.tensor.matmul(
                        acc[:, E:2 * E], xc[:, 1, c, :], wt[:, c, 0:E],
                        start=False, stop=(c == KC - 1))

            def routing(blk, acc):
                # comb = corr/S + main (exact: 1/S is a power of two).
                # an instruction may read at most one PSUM operand, so the
                # corr half is scaled out to SBUF on ACT first.
                corr_s = rt.tile([128, E], F32, tag="corr_s")
                nc.scalar.activation(corr_s[:], acc[:, E:2 * E], AF.Copy,
                                     scale=1.0 / S)
                comb = rt.tile([128, E], F32, tag="comb")
                nc.vector.scalar_tensor_tensor(
                    comb[:], acc[:, 0:E], 1.0, corr_s[:],
                    ALU.mult, ALU.add)

                scores = rt.tile([128, E], F32, tag="scores")
                nc.scalar.activation(scores[:], comb[:], AF.Sigmoid)

                s4c = rt.tile([128, E], F32, tag="s4c")
                nc.vector.tensor_tensor(s4c[:], scores[:], bias_t[:], ALU.add)

                m1 = rt.tile([128, G], F32, tag="m1")
                nc.vector.reduce_max(
                    m1[:], s4c[:].rearrange("p (g s) -> p g s", g=G),
                    axis=mybir.AxisListType.X)
                s4cr = rt.tile([128, E], F32, tag="s4cr")
                nc.vector.match_replace(s4cr[:], m1[:], s4c[:], -1e30)
                m2 = rt.tile([128, G], F32, tag="m2")
                nc.vector.reduce_max(
                    m2[:], s4cr[:].rearrange("p (g s) -> p g s", g=G),
                    axis=mybir.AxisListType.X)
                gsc = rt.tile([128, G], F32, tag="gsc")
                nc.vector.tensor_tensor(gsc[:], m1[:], m2[:], ALU.add)

                gsorted = rt.tile([128, 8], F32, tag="gsorted")
                nc.vector.max(gsorted[:], gsc[:])
                gmask = rt.tile([128, G], F32, tag="gmask")
                nc.vector.tensor_scalar(
                    gmask[:], gsc[:], gsorted[:, 3:4], None, ALU.is_ge)

                masked = rt.tile([128, E], F32, tag="masked")
                nc.vector.tensor_tensor(
                    masked[:].rearrange("p (g s) -> p g s", g=G),
                    s4c[:].rearrange("p (g s) -> p g s", g=G),
                    gmask[:].unsqueeze(-1).broadcast_to([128, G, GS]),
                    ALU.mult)

                vals = rt.tile([128, 8], F32, tag="vals")
                nc.vector.max(vals[:], masked[:])
                idx16 = rt.tile([128, 8], U16, tag="idx16")
                nc.vector.max_index(idx16[:], vals[:], masked[:])

                # index output leaves early (on the ACT HWDGE ring, which is
                # idle after the weights) so its DMA latency hides under the
                # gather stage and never head-of-line blocks the x prefetch
                iout = outp.tile([128, TOP_K], I32, tag="iout")
                nc.vector.tensor_copy(iout[:], idx16[:])
                t0 = blk * TB
                nc.scalar.dma_start(idx_out[t0:t0 + TB, :], iout[:])

                # per-slot gather scores[idx[k]]: match idx against an
                # iota row (unique values -> tie-safe), accumulate
                idxf = rt.tile([128, 8], F32, tag="idxf")
                nc.vector.tensor_copy(idxf[:], idx16[:])
                w8 = rt.tile([128, 8], F32, tag="w8")
                scr_v = rt.tile([128, E], F32, tag="scr_v")
                for k in range(TOP_K):
                    nc.vector.scalar_tensor_tensor(
                        scr_v[:], iota_t[:], idxf[:, k:k + 1], scores[:],
                        ALU.is_equal, ALU.mult,
                        accum_out=w8[:, k:k + 1])

                denom = rt.tile([128, 1], F32, tag="denom")
                nc.vector.reduce_sum(denom[:], w8[:], axis=mybir.AxisListType.X)
                rec = rt.tile([128, 1], F32, tag="rec")
                nc.vector.tensor_scalar_add(denom[:], denom[:], 1e-20)
                nc.vector.reciprocal(rec[:], denom[:])
                nc.vector.tensor_scalar_mul(rec[:], rec[:], ROUTED_SCALING)

                wout = outp.tile([128, TOP_K], F32, tag="wout")
                nc.vector.tensor_scalar(
                    wout[:], w8[:], rec[:, 0:1], None, ALU.mult)
                nc.scalar.dma_start(w_out[t0:t0 + TB, :], wout[:])

            def body():
                # blocks 0-1 run pass-reordered: both main passes first, then
                # the lo-x passes, then the wl-dependent h*wl passes, so the
                # in-order PE queue never stalls on late-streaming operands
                # (the x ring delivers x0h, x1h, x0l, x1l in need order and
                # the weight ring delivers wh before wl).
                xc0 = alloc_block(0)
                dma_hi(0, xc0, sliced=True)
                xc1 = alloc_block(1)
                dma_hi(1, xc1)
                dma_lo(0, xc0)
                dma_lo(1, xc1)
                acc0 = ps.tile([128, 2 * E], F32, tag="acc", name="acc0")
                acc1 = ps.tile([128, 2 * E], F32, tag="acc", name="acc1")
                mm_main_pass(xc0, acc0)
                mm_main_pass(xc1, acc1)
                mm_lwh_pass(xc0, acc0)
                mm_lwh_pass(xc1, acc1)
                mm_hwl_pass(xc0, acc0)
                routing(0, acc0)
                mm_hwl_pass(xc1, acc1)
                routing(1, acc1)
                for blk in range(2, NB):
                    xc = dma_block(blk)
                    acc = ps.tile([128, 2 * E], F32, tag="acc", name=f"acc{blk}")
                    mm_fused(xc, acc)
                    routing(blk, acc)

            if reps == 1:
                body()
            else:
                assert reps % unroll == 0
                with tc.For_i(0, reps // unroll, 1,
                              staggered_reset=staggered_reset):
                    for _ in range(unroll):
                        body()
    nc.compile()
    return nc


_PROGRAM_CACHE = {}


def _get_program(reps=1):
    if reps not in _PROGRAM_CACHE:
        _PROGRAM_CACHE[reps] = build_program(reps)
    return _PROGRAM_CACHE[reps]


_F16_MIN_NORMAL = 2.0 ** -14


def _split_f16(x):
    """x (f32) -> (h, l) fp16 planes with x ~= h + l/S; subnormals zeroed."""
    h = x.astype(np.float16)
    h32 = h.astype(np.float32)
    h = np.where(np.abs(h32) < _F16_MIN_NORMAL, np.float16(0), h)
    h32 = h.astype(np.float32)
    l = ((x - h32) * np.float32(S)).astype(np.float16)
    l32 = l.astype(np.float32)
    l = np.where(np.abs(l32) < _F16_MIN_NORMAL, np.float16(0), l)
    return h, l


def _blockify(plane_t):
    """[H, T_CORE] -> [NB, 128, KC, TB] so per-block partition lines are contiguous."""
    # element (h, t): h = c*128 + p, t = blk*TB + tb -> out[blk, p, c, tb]
    a = plane_t.reshape(KC, 128, NB, TB)       # [c, p, blk, tb]
    return np.ascontiguousarray(a.transpose(2, 1, 0, 3))


def _prepare_inputs(hidden_states, weight, e_score_correction_bias):
    hs = np.asarray(hidden_states, dtype=np.float32)
    w = np.asarray(weight, dtype=np.float32)
    b = np.asarray(e_score_correction_bias, dtype=np.float32)

    wh, wl = _split_f16(w)
    # [p, c, e] layout per plane, then concat on the e axis -> [p, c, 2e]
    wt_h = wh.T.reshape(KC, 128, E).transpose(1, 0, 2)
    wt_l = wl.T.reshape(KC, 128, E).transpose(1, 0, 2)
    wt_hl = np.ascontiguousarray(np.concatenate([wt_h, wt_l], axis=2))
    bias_bc = np.ascontiguousarray(np.broadcast_to(b, (128, E)))
    iota_bc = np.ascontiguousarray(
        np.broadcast_to(np.arange(E, dtype=np.float32), (128, E)))

    in_maps = []
    for c in range(N_CORES):
        sl = hs[c * T_CORE:(c + 1) * T_CORE]  # [T_CORE, H]
        h, l = _split_f16(sl)
        bh = _blockify(np.ascontiguousarray(h.T))
        bl = _blockify(np.ascontiguousarray(l.T))
        in_maps.append({
            # [blk, p, plane, c, tb]
            "hst_c": np.ascontiguousarray(
                np.stack([bh, bl], axis=2)),
            "wt_hl": wt_hl,
            "bias_bc": bias_bc,
            "iota_bc": iota_bc,
        })
    return in_maps


def kernel(hidden_states, weight, e_score_correction_bias):
    in_maps = _prepare_inputs(hidden_states, weight, e_score_correction_bias)
    nc = _get_program(1)
    res = run_bass_kernel_spmd(nc, in_maps, list(range(N_CORES)))
    idx = np.concatenate([r["idx_out"] for r in res.results], axis=0)
    w = np.concatenate([r["w_out"] for r in res.results], axis=0)
    return idx.astype(np.int32), w.astype(np.float32)


# revision 19
# speedup vs baseline: 1.1334x; 1.1334x over previous
"""NemotronH Top-k MoE router on 8 Trainium2 NeuronCores.

Token-parallel: 16384 tokens sharded 2048/core; router weight replicated.

Matmul in [t, e] layout with a fused moving operand: for each 128-token
block, PSUM out[t=128, 512] holds main logits in cols 0:256 and the
correction in cols 256:512.  Pass 1 streams the combined [wh|wl] weight
(N=512) with the hi-plane chunk stationary (computes h*wh and h*wl in one
sweep); pass 2 streams wh (N=256) with the lo-plane chunk stationary and
accumulates l*wh onto the correction half.  Same FLOPs as a 3-pass
scheme but 2/3 the LDWEIGHTS and one PSUM bank per block.  Block 0 runs
unfused (wh-only passes first) so the PE can start before the lo-plane
weights arrive; weight DMA streams the wh plane first for the same
reason, and block DMAs are plane-split so a block's main pass only waits
on its hi half.

Precision: fp16 hi/lo planes, x = h + l/2048 (22 mantissa bits); products
in fp22, fp32 PSUM accumulation; logits = main + corr/2048.  All
selection comparisons run in fp32.

Routing per block (tokens on partitions, experts on free axis): sigmoid,
bias add, group top-2 via reduce_max + match_replace, group top-4 via
max8 threshold, top-8 via max8 + max_index, per-slot score extraction via
is_equal(iota) + accumulate, normalize, scale.

Timing programs (reps > 1) unroll the loop body 2x so the For_i
all-engine barrier only fires every other invocation and consecutive
bodies pipeline across the seam.
"""
import sys
sys.path.insert(0, "/opt/trn_rl_repo")

import numpy as np

from concourse import bacc, tile, mybir
from concourse.bass_utils import run_bass_kernel_spmd

F32 = mybir.dt.float32
F16 = mybir.dt.float16
U16 = mybir.dt.uint16
I32 = mybir.dt.int32
AF = mybir.ActivationFunctionType
ALU = mybir.AluOpType

T_TOTAL = 16384
H = 4096
E = 256
G, GS = 8, 32
TOP_K = 8
N_CORES = 8
T_CORE = T_TOTAL // N_CORES      # 2048
TB = 128                         # tokens per block (PSUM partition dim)
NB = T_CORE // TB                # 16 blocks
KC = H // 128                    # 32 k-chunks
S = 2048.0                       # lo-plane scale (2^11)
ROUTED_SCALING = 2.5


def build_program(reps=1, staggered_reset=False, unroll=1):
    nc = bacc.Bacc("TRN2", target_bir_lowering=False)
    # host pre-layout: [blk, p, plane, c, tb]; per-plane partition lines are
    # 8 KB contiguous so hi/lo can stream as separate DMAs
    hst_c = nc.dram_tensor("hst_c", [NB, 128, 2, KC, TB], F16, kind="ExternalInput")
    # combined weights [p, c, (wh|wl)]
    wt_d = nc.dram_tensor("wt_hl", [128, KC, 2 * E], F16, kind="ExternalInput")
    bias_d = nc.dram_tensor("bias_bc", [128, E], F32, kind="ExternalInput")
    iota_d = nc.dram_tensor("iota_bc", [128, E], F32, kind="ExternalInput")
    idx_out = nc.dram_tensor("idx_out", [T_CORE, TOP_K], I32, kind="ExternalOutput")
    w_out = nc.dram_tensor("w_out", [T_CORE, TOP_K], F32, kind="ExternalOutput")

    with tile.TileContext(nc) as tc:
        with (
            tc.tile_pool(name="const", bufs=1) as cpool,
            tc.tile_pool(name="hs", bufs=4) as hspool,
            tc.tile_pool(name="rt", bufs=3) as rt,
            tc.tile_pool(name="outp", bufs=2) as outp,
            tc.tile_pool(name="ps", bufs=4, space="PSUM") as ps,
        ):
            wt = cpool.tile([128, KC, 2 * E], F16, name="wt")
            bias_t = cpool.tile([128, E], F32)
            iota_t = cpool.tile([128, E], F32)
            # weights on the ACT HWDGE ring (hidden goes on the SP ring).
            # hi plane first in chunk slices (512 B runs -> full-rate DMA),
            # then bias/iota, then the lo plane: block 0 is scheduled to not
            # need wl until ~2 passes in.
            for c0, c1 in ((0, 2), (2, 8), (8, 20), (20, KC)):
                nc.scalar.dma_start(wt[:, c0:c1, 0:E], wt_d[:, c0:c1, 0:E])
            nc.scalar.dma_start(bias_t[:], bias_d[:])
            for c0, c1 in ((0, 8), (8, 20), (20, KC)):
                nc.scalar.dma_start(wt[:, c0:c1, E:2 * E], wt_d[:, c0:c1, E:2 * E])
            nc.scalar.dma_start(iota_t[:], iota_d[:])

            def alloc_block(blk):
                return hspool.tile([128, 2, KC, TB], F16, tag="xc",
                                   name=f"xc{blk}")

            def dma_hi(blk, xc, sliced=False):
                if sliced:
                    for c0, c1 in ((0, 2), (2, 8), (8, 20), (20, KC)):
                        nc.sync.dma_start(xc[:, 0, c0:c1, :],
                                          hst_c[blk, :, 0, c0:c1, :])
                else:
                    nc.sync.dma_start(xc[:, 0], hst_c[blk, :, 0])

            def dma_lo(blk, xc):
                nc.sync.dma_start(xc[:, 1], hst_c[blk, :, 1])

            def dma_block(blk):
                xc = alloc_block(blk)
                dma_hi(blk, xc)
                dma_lo(blk, xc)
                return xc

            def mm_main_pass(xc, acc):
                # main into cols 0:256 (wh only).  per-element has_written
                # bits make the first write to each region an overwrite.
                for c in range(KC):
                    nc.tensor.matmul(
                        acc[:, 0:E], xc[:, 0, c, :], wt[:, c, 0:E],
                        start=(c == 0), stop=False)

            def mm_lwh_pass(xc, acc):
                for c in range(KC):
                    nc.tensor.matmul(
                        acc[:, E:2 * E], xc[:, 1, c, :], wt[:, c, 0:E],
                        start=False, stop=False)

            def mm_hwl_pass(xc, acc):
                for c in range(KC):
                    nc.tensor.matmul(
                        acc[:, E:2 * E], xc[:, 0, c, :], wt[:, c, E:2 * E],
                        start=False, stop=(c == KC - 1))

            def mm_fused(xc, acc):
                # pass 1: acc[:, 0:256] += xh_c.T @ wh_c
                #         acc[:, 256:512] += xh_c.T @ wl_c
                for c in range(KC):
                    nc.tensor.matmul(
                        acc[:, 0:2 * E], xc[:, 0, c, :], wt[:, c, :],
                        start=(c == 0), stop=False)
                # pass 2: acc[:, 256:512] += xl_c.T @ wh_c
                for c in range(KC):
                    nc.tensor.matmul(
                        acc[:, E:2 * E], xc[:, 1, c, :], wt[:, c, 0:E],
                        start=False, stop=(c == KC - 1))

            def routing(blk, acc):
                # comb = corr/S + main (exact: 1/S is a power of two).
                # an instruction may read at most one PSUM operand, so the
                # corr half is scaled out to SBUF on ACT first.
                corr_s = rt.tile([128, E], F32, tag="corr_s")
                nc.scalar.activation(corr_s[:], acc[:, E:2 * E], AF.Copy,
                                     scale=1.0 / S)
                comb = rt.tile([128, E], F32, tag="comb")
                nc.vector.scalar_tensor_tensor(
                    comb[:], acc[:, 0:E], 1.0, corr_s[:],
                    ALU.mult, ALU.add)

                scores = rt.tile([128, E], F32, tag="scores")
                nc.scalar.activation(scores[:], comb[:], AF.Sigmoid)

                s4c = rt.tile([128, E], F32, tag="s4c")
                nc.vector.tensor_tensor(s4c[:], scores[:], bias_t[:], ALU.add)

                m1 = rt.tile([128, G], F32, tag="m1")
                nc.vector.reduce_max(
                    m1[:], s4c[:].rearrange("p (g s) -> p g s", g=G),
                    axis=mybir.AxisListType.X)
                s4cr = rt.tile([128, E], F32, tag="s4cr")
                nc.vector.match_replace(s4cr[:], m1[:], s4c[:], -1e30)
                m2 = rt.tile([128, G], F32, tag="m2")
                nc.vector.reduce_max(
                    m2[:], s4cr[:].rearrange("p (g s) -> p g s", g=G),
                    axis=mybir.AxisListType.X)
                gsc = rt.tile([128, G], F32, tag="gsc")
                nc.vector.tensor_tensor(gsc[:], m1[:], m2[:], ALU.add)

                gsorted = rt.tile([128, 8], F32, tag="gsorted")
                nc.vector.max(gsorted[:], gsc[:])
                gmask = rt.tile([128, G], F32, tag="gmask")
                nc.vector.tensor_scalar(
                    gmask[:], gsc[:], gsorted[:, 3:4], None, ALU.is_ge)

                masked = rt.tile([128, E], F32, tag="masked")
                nc.vector.tensor_tensor(
                    masked[:].rearrange("p (g s) -> p g s", g=G),
                    s4c[:].rearrange("p (g s) -> p g s", g=G),
                    gmask[:].unsqueeze(-1).broadcast_to([128, G, GS]),
                    ALU.mult)

                vals = rt.tile([128, 8], F32, tag="vals")
                nc.vector.max(vals[:], masked[:])
                idx16 = rt.tile([128, 8], U16, tag="idx16")
                nc.vector.max_index(idx16[:], vals[:], masked[:])

                # index output leaves early (on the ACT HWDGE ring, which is
                # idle after the weights) so its DMA latency hides under the
                # gather stage and never head-of-line blocks the x prefetch
                iout = outp.tile([128, TOP_K], I32, tag="iout")
                nc.vector.tensor_copy(iout[:], idx16[:])
                t0 = blk * TB
                nc.scalar.dma_start(idx_out[t0:t0 + TB, :], iout[:])

                # per-slot gather scores[idx[k]]: match idx against an
                # iota row (unique values -> tie-safe), accumulate
                idxf = rt.tile([128, 8], F32, tag="idxf")
                nc.vector.tensor_copy(idxf[:], idx16[:])
                w8 = rt.tile([128, 8], F32, tag="w8")
                scr_v = rt.tile([128, E], F32, tag="scr_v")
                for k in range(TOP_K):
                    nc.vector.scalar_tensor_tensor(
                        scr_v[:], iota_t[:], idxf[:, k:k + 1], scores[:],
                        ALU.is_equal, ALU.mult,
                        accum_out=w8[:, k:k + 1])

                denom = rt.tile([128, 1], F32, tag="denom")
                nc.vector.reduce_sum(denom[:], w8[:], axis=mybir.AxisListType.X)
                rec = rt.tile([128, 1], F32, tag="rec")
                nc.vector.tensor_scalar_add(denom[:], denom[:], 1e-20)
                nc.vector.reciprocal(rec[:], denom[:])
                nc.vector.tensor_scalar_mul(rec[:], rec[:], ROUTED_SCALING)

                wout = outp.tile([128, TOP_K], F32, tag="wout")
                nc.vector.tensor_scalar(
                    wout[:], w8[:], rec[:, 0:1], None, ALU.mult)
                nc.scalar.dma_start(w_out[t0:t0 + TB, :], wout[:])

            def body():
                # blocks 0-1 run pass-reordered: both main passes first, then
                # the lo-x passes, then the wl-dependent h*wl passes, so the
                # in-order PE queue never stalls on late-streaming operands
                # (the x ring delivers x0h, x1h, x0l, x1l in need order and
                # the weight ring delivers wh before wl).
                xc0 = alloc_block(0)
                dma_hi(0, xc0, sliced=True)
                xc1 = alloc_block(1)
                dma_hi(1, xc1)
                dma_lo(0, xc0)
                dma_lo(1, xc1)
                acc0 = ps.tile([128, 2 * E], F32, tag="acc", name="acc0")
                acc1 = ps.tile([128, 2 * E], F32, tag="acc", name="acc1")
                mm_main_pass(xc0, acc0)
                mm_main_pass(xc1, acc1)
                mm_lwh_pass(xc0, acc0)
                mm_lwh_pass(xc1, acc1)
                mm_hwl_pass(xc0, acc0)
                routing(0, acc0)
                mm_hwl_pass(xc1, acc1)
                routing(1, acc1)
                for blk in range(2, NB):
                    xc = dma_block(blk)
                    acc = ps.tile([128, 2 * E], F32, tag="acc", name=f"acc{blk}")
                    mm_fused(xc, acc)
                    routing(blk, acc)

            if reps == 1:
                body()
            else:
                assert reps % unroll == 0
                with tc.For_i(0, reps // unroll, 1,
                              staggered_reset=staggered_reset):
                    for _ in range(unroll):
                        body()
    nc.compile()
    return nc


_PROGRAM_CACHE = {}


def _get_program(reps=1):
    if reps not in _PROGRAM_CACHE:
        _PROGRAM_CACHE[reps] = build_program(reps)
    return _PROGRAM_CACHE[reps]


_F16_MIN_NORMAL = 2.0 ** -14


def _split_f16(x):
    """x (f32) -> (h, l) fp16 planes with x ~= h + l/S; subnormals zeroed."""
    h = x.astype(np.float16)
    h32 = h.astype(np.float32)
    h = np.where(np.abs(h32) < _F16_MIN_NORMAL, np.float16(0), h)
    h32 = h.astype(np.float32)
    l = ((x - h32) * np.float32(S)).astype(np.float16)
    l32 = l.astype(np.float32)
    l = np.where(np.abs(l32) < _F16_MIN_NORMAL, np.float16(0), l)
    return h, l


def _blockify(plane_t):
    """[H, T_CORE] -> [NB, 128, KC, TB] so per-block partition lines are contiguous."""
    # element (h, t): h = c*128 + p, t = blk*TB + tb -> out[blk, p, c, tb]
    a = plane_t.reshape(KC, 128, NB, TB)       # [c, p, blk, tb]
    return np.ascontiguousarray(a.transpose(2, 1, 0, 3))


def _prepare_inputs(hidden_states, weight, e_score_correction_bias):
    hs = np.asarray(hidden_states, dtype=np.float32)
    w = np.asarray(weight, dtype=np.float32)
    b = np.asarray(e_score_correction_bias, dtype=np.float32)

    wh, wl = _split_f16(w)
    # [p, c, e] layout per plane, then concat on the e axis -> [p, c, 2e]
    wt_h = wh.T.reshape(KC, 128, E).transpose(1, 0, 2)
    wt_l = wl.T.reshape(KC, 128, E).transpose(1, 0, 2)
    wt_hl = np.ascontiguousarray(np.concatenate([wt_h, wt_l], axis=2))
    bias_bc = np.ascontiguousarray(np.broadcast_to(b, (128, E)))
    iota_bc = np.ascontiguousarray(
        np.broadcast_to(np.arange(E, dtype=np.float32), (128, E)))

    in_maps = []
    for c in range(N_CORES):
        sl = hs[c * T_CORE:(c + 1) * T_CORE]  # [T_CORE, H]
        h, l = _split_f16(sl)
        bh = _blockify(np.ascontiguousarray(h.T))
        bl = _blockify(np.ascontiguousarray(l.T))
        in_maps.append({
            # [blk, p, plane, c, tb]
            "hst_c": np.ascontiguousarray(
                np.stack([bh, bl], axis=2)),
            "wt_hl": wt_hl,
            "bias_bc": bias_bc,
            "iota_bc": iota_bc,
        })
    return in_maps


def kernel(hidden_states, weight, e_score_correction_bias):
    in_maps = _prepare_inputs(hidden_states, weight, e_score_correction_bias)
    nc = _get_program(1)
    res = run_bass_kernel_spmd(nc, in_maps, list(range(N_CORES)))
    idx = np.concatenate([r["idx_out"] for r in res.results], axis=0)
    w = np.concatenate([r["w_out"] for r in res.results], axis=0)
    return idx.astype(np.int32), w.astype(np.float32)
